# revision 10
# baseline (speedup 1.0000x reference)
"""GQA causal attention (B=2,T=2048,C=2048,H=32,HKV=8,D=64) on 8 TRN2 cores.

Sharding: tensor-parallel over GQA groups — core c owns q heads 4c..4c+3 and
kv head c. Each core computes its 4-head attention and a partial c_proj
against wc[:, 256c:256c+256]; the host sums the 8 partials (all-reduce).

Per-core kernel layout (everything transposed so contraction dims sit on
SBUF partitions, avoiding on-chip transposes of activations):
  qT[m,t] via lhsT=wqT[c,m], rhs=xT[c,t]      (bf16 matmul, fp32 psum)
  RoPE in [d,t] layout: rot(q) done with a constant permutation matmul
  S^T[j,i] matmuls with K=d=64; even/odd heads use partition halves
  0:64 / 64:128 so pairs row-pack in the PE array
  exp via ACT over 2-bank PSUM pairs (scale=1/sqrt(D) folded in),
  causal mask via 0/1 pattern multiply
  y'^T[65,i] = v'Seq.T @ expS^T with an ones-column giving softmax sums
  divide via reciprocal + PE ones-broadcast
  c_proj is interleaved into the attention i-block loop to keep PE fed
"""

import math
import numpy as np

B, T, C = 2, 2048, 2048
H, HKV, D = 32, 8, 64
NCORES = 8
QS = (H // NCORES) * D  # 256 q-proj cols per core
P = 128
BT = B * T
CO = C // P  # 16 contraction chunks
NB = T // 512  # 4 i-blocks per batch

_CACHE = {}


def _build_program():
    import concourse.bass as bass
    import concourse.mybir as mybir
    import concourse.tile as tile
    from concourse import bacc

    f32 = mybir.dt.float32
    bf16 = mybir.dt.bfloat16
    AF = mybir.ActivationFunctionType

    nc = bacc.Bacc("TRN2", target_bir_lowering=False, debug=False)

    xT_d = nc.declare_dram_parameter("xT", [P, CO, BT], bf16, isOutput=False)
    wq_d = nc.declare_dram_parameter("wqT", [P, CO, QS], bf16, isOutput=False)
    wkv_d = nc.declare_dram_parameter("wkvT", [P, CO, P], bf16, isOutput=False)
    wc_d = nc.declare_dram_parameter("wcT", [P, 2, C], bf16, isOutput=False)
    cs_d = nc.declare_dram_parameter("cs", [P, 2, T], bf16, isOutput=False)
    rot_d = nc.declare_dram_parameter("rotT", [P, P], bf16, isOutput=False)
    idn_d = nc.declare_dram_parameter("idn", [P, P], bf16, isOutput=False)
    mp_d = nc.declare_dram_parameter("maskpat", [P, 4, 1024], bf16, isOutput=False)
    out_d = nc.declare_dram_parameter("partial", [BT, C], bf16, isOutput=True)

    with tile.TileContext(nc) as tc:
        with (
            tc.tile_pool(name="const", bufs=1) as cpool,
            tc.tile_pool(name="res", bufs=1) as rpool,
            tc.tile_pool(name="work", bufs=3) as wpool,
            tc.tile_pool(name="exps", bufs=12) as epool,
            tc.tile_pool(name="psum", bufs=8, space="PSUM") as ppool,
        ):
            # resident constants
            wq_sb = cpool.tile([P, CO, QS], bf16)
            nc.sync.dma_start(wq_sb[:], wq_d[:])
            wkv_sb = cpool.tile([P, CO, P], bf16)
            nc.sync.dma_start(wkv_sb[:], wkv_d[:])
            wc_sb = cpool.tile([P, 2, C], bf16)
            nc.sync.dma_start(wc_sb[:], wc_d[:])
            cs_sb = cpool.tile([P, 2, T], bf16)
            nc.sync.dma_start(cs_sb[:], cs_d[:])
            rot_sb = cpool.tile([P, P], bf16)
            nc.sync.dma_start(rot_sb[:], rot_d[:])
            idn_sb = cpool.tile([P, P], bf16)
            nc.sync.dma_start(idn_sb[:], idn_d[:])
            mp_sb = cpool.tile([P, 4, 1024], bf16)
            nc.sync.dma_start(mp_sb[:], mp_d[:])
            ones_sb = cpool.tile([65, 64], bf16)
            nc.vector.memset(ones_sb[64:65, :], 1.0)

            # resident per-batch activations
            qT_sb = rpool.tile([P, 2, T], bf16)  # [qb+d, mtile, t]
            kT2_sb = rpool.tile([P, T], bf16)  # k^T duplicated on both halves
            vT_sb = rpool.tile([P, T], bf16)  # v^T on partitions 64:128
            vseq_sb = rpool.tile([P, CO, 65], bf16)  # [j_local, jc, d|ones]
            yT_sb = rpool.tile([P, 2, T], bf16)

            nc.vector.memset(vseq_sb[:, :, 64:65], 1.0)

            def attn_pair(mt, ib, t0):
                # heads 2*mt (partitions 0:64) and 2*mt+1 (64:128) together:
                # one [128,1024] scores psum per jc, one exp, row-packed MMs
                isl = slice(ib * 512, (ib + 1) * 512)
                njc = 4 * (ib + 1)
                pvE = ppool.tile([P, 512], f32, tag="pv", bufs=2, name=f"pvE{mt}_{ib}")
                pvO = ppool.tile([P, 512], f32, tag="pv", bufs=2, name=f"pvO{mt}_{ib}")
                for jc in range(njc):
                    sps = ppool.tile(
                        [P, 1024], f32, tag="spair", bufs=2, name=f"sp{mt}_{ib}_{jc}"
                    )
                    for sh in range(2):
                        qb = sh * 64
                        nc.tensor.matmul(
                            sps[:, sh * 512 : (sh + 1) * 512],
                            kT2_sb[qb : qb + 64, jc * P : (jc + 1) * P],
                            qT_sb[qb : qb + 64, mt, isl],
                            start=True,
                            stop=True,
                        )
                    et = epool.tile([P, 1024], bf16, tag="expS", name=f"et{mt}_{ib}_{jc}")
                    nc.scalar.activation(et[:], sps[:], AF.Exp, scale=1.0 / math.sqrt(D))
                    if jc >= 4 * ib:
                        nc.vector.tensor_mul(et[:], et[:], mp_sb[:, jc - 4 * ib, :])
                    for sh, pv in ((0, pvE), (1, pvO)):
                        nc.tensor.matmul(
                            pv[0:65, :],
                            vseq_sb[:, jc, :],
                            et[:, sh * 512 : (sh + 1) * 512],
                            start=(jc == 0),
                            stop=(jc == njc - 1),
                        )
                for sh, pv in ((0, pvE), (1, pvO)):
                    pvs = wpool.tile([65, 512], f32, tag="pvs")
                    nc.vector.tensor_copy(pvs[:], pv[0:65, :])
                    rec = wpool.tile([65, 512], bf16, tag="rec")
                    with nc.allow_low_precision(reason="softmax recip in bf16"):
                        nc.vector.reciprocal(rec[64:65, :], pvs[64:65, :])
                    bc = ppool.tile([P, 512], f32, tag="pv", bufs=2, name=f"bc{mt}_{ib}_{sh}")
                    nc.tensor.matmul(
                        bc[0:64, :], ones_sb[64:65, :], rec[64:65, :], start=True, stop=True
                    )
                    if sh == 0:
                        nc.vector.tensor_mul(yT_sb[0:64, mt, isl], pvs[0:64, :], bc[0:64, :])
                    else:
                        yt = wpool.tile([64, 512], bf16, tag="ytmp")
                        nc.vector.tensor_mul(yt[:], pvs[0:64, :], bc[0:64, :])
                        nc.sync.dma_start(yT_sb[64:P, mt, isl], yt[:])

            def cproj_chunk(tcn, t0):
                co = wpool.tile([P, C], bf16, tag="cpo")
                for nb in range(4):
                    cps = ppool.tile(
                        [P, 512], f32, tag="ps", bufs=2, name=f"cp{tcn}_{nb}"
                    )
                    for m in range(2):
                        nc.tensor.matmul(
                            cps[:],
                            yT_sb[:, m, tcn * P : (tcn + 1) * P],
                            wc_sb[:, m, nb * 512 : (nb + 1) * 512],
                            start=(m == 0),
                            stop=(m == 1),
                        )
                    if nb % 2 == 0:
                        nc.vector.tensor_copy(co[:, nb * 512 : (nb + 1) * 512], cps[:])
                    else:
                        nc.scalar.copy(co[:, nb * 512 : (nb + 1) * 512], cps[:])
                nc.sync.dma_start(out_d[t0 + tcn * P : t0 + (tcn + 1) * P, :], co[:])

            for bi in range(B):
                t0 = bi * T
                # ---- projections + RoPE ----
                for tq in range(4):
                    tsl = slice(tq * 512, (tq + 1) * 512)
                    x_sb = wpool.tile([P, CO, 512], bf16, tag="x", bufs=2)
                    nc.sync.dma_start(
                        x_sb[:], xT_d[:, :, t0 + tq * 512 : t0 + (tq + 1) * 512]
                    )
                    for mt in range(3):
                        ps = ppool.tile([P, 512], f32, tag="ps", bufs=2)
                        for o in range(CO):
                            lhsT = (
                                wq_sb[:, o, mt * P : (mt + 1) * P]
                                if mt < 2
                                else wkv_sb[:, o, :]
                            )
                            nc.tensor.matmul(
                                ps[:],
                                lhsT,
                                x_sb[:, o, :],
                                start=(o == 0),
                                stop=(o == CO - 1),
                            )
                        if mt < 2:  # q heads: RoPE, out bf16
                            qraw = wpool.tile([P, 512], bf16, tag="qraw")
                            nc.scalar.copy(qraw[:], ps[:])
                            rps = ppool.tile([P, 512], f32, tag="ps", bufs=2)
                            nc.tensor.matmul(
                                rps[:], rot_sb[:], qraw[:], start=True, stop=True
                            )
                            t1 = wpool.tile([P, 512], f32, tag="t1")
                            nc.vector.tensor_mul(t1[:], qraw[:], cs_sb[:, 0, tsl])
                            t2 = wpool.tile([P, 512], f32, tag="t2")
                            nc.vector.tensor_mul(t2[:], rps[:], cs_sb[:, 1, tsl])
                            nc.vector.tensor_add(qT_sb[:, mt, tsl], t1[:], t2[:])
                        else:  # kv tile: rope k (rows 0:64), copy v (rows 64:128)
                            kraw = wpool.tile([64, 512], bf16, tag="kraw")
                            nc.scalar.copy(kraw[:], ps[0:64, :])
                            rps = ppool.tile([P, 512], f32, tag="ps", bufs=2)
                            nc.tensor.matmul(
                                rps[0:64, :],
                                rot_sb[0:64, 0:64],
                                kraw[:],
                                start=True,
                                stop=True,
                            )
                            tk1 = wpool.tile([64, 512], f32, tag="tk1")
                            nc.vector.tensor_mul(tk1[:], kraw[:], cs_sb[0:64, 0, tsl])
                            tk2 = wpool.tile([64, 512], f32, tag="tk2")
                            nc.vector.tensor_mul(
                                tk2[:], rps[0:64, :], cs_sb[0:64, 1, tsl]
                            )
                            nc.vector.tensor_add(kT2_sb[0:64, tsl], tk1[:], tk2[:])
                            nc.scalar.copy(vT_sb[64:P, tsl], ps[64:P, :])
                # duplicate k^T onto partitions 64:128 (for odd-head row packing)
                nc.sync.dma_start(kT2_sb[64:P, :], kT2_sb[0:64, :])

                # ---- v' in sequence-major layout via PE transpose ----
                for tcn in range(CO):
                    tp = ppool.tile([P, 512], bf16, tag="ps", bufs=2)
                    nc.tensor.transpose(
                        tp[:, 0:64],
                        vT_sb[64:P, tcn * P : (tcn + 1) * P],
                        idn_sb[64:P, 64:P],
                    )
                    nc.vector.tensor_copy(vseq_sb[:, tcn, 0:64], tp[:, 0:64])

                # ---- attention + c_proj interleaved per i-block ----
                for ib in range(NB):
                    for mt in range(2):
                        attn_pair(mt, ib, t0)
                    for tcn in range(4 * ib, 4 * ib + 4):
                        cproj_chunk(tcn, t0)
    nc.compile()
    return nc


def _host_inputs(x, wq, wk, wv, wc):
    import ml_dtypes

    bfl = ml_dtypes.bfloat16

    def chunk_pfirst(a):  # [C_like, M] -> [P, C_like//P, M], c = o*P + p
        c, m = a.shape
        return np.ascontiguousarray(a.reshape(c // P, P, m).transpose(1, 0, 2))

    xT = np.ascontiguousarray(x.reshape(BT, C).T)  # [C, BT]
    xT_h = chunk_pfirst(xT).astype(bfl)

    # RoPE tables, transposed: [d, t], two heads stacked
    inv = 1.0 / (10000.0 ** (np.arange(0, D, 2, dtype=np.float64) / D))
    pos = np.arange(T, dtype=np.float64)
    emb = np.concatenate([pos[:, None] * inv[None, :]] * 2, axis=1)  # [T, D]
    cosT = np.cos(emb).T.astype(np.float32)  # [D, T]
    sinT = np.sin(emb).T.astype(np.float32)
    cs = np.zeros((P, 2, T), np.float32)
    cs[0:64, 0], cs[64:128, 0] = cosT, cosT
    cs[0:64, 1], cs[64:128, 1] = sinT, sinT
    cs_h = cs.astype(bfl)

    # rot(q)[dout] = sum_din R[dout,din] q[din]; lhsT = R.T
    R = np.zeros((D, D), np.float32)
    for d in range(32):
        R[d, d + 32] = -1.0
        R[d + 32, d] = 1.0
    R2 = np.zeros((P, P), np.float32)
    R2[0:64, 0:64], R2[64:128, 64:128] = R, R
    rot_h = np.ascontiguousarray(R2.T).astype(bfl)

    idn_h = np.eye(P, dtype=np.float32).astype(bfl)

    # causal patterns for diagonal-straddling S^T blocks: keep j <= i
    dj = np.arange(P)[:, None]
    di = np.arange(512)[None, :]
    mp = np.zeros((P, 4, 1024), np.float32)
    for p in range(4):
        pat = (di >= p * P + dj).astype(np.float32)
        mp[:, p, 0:512] = pat
        mp[:, p, 512:1024] = pat
    mp_h = mp.astype(bfl)

    per_core = []
    for core in range(NCORES):
        qs = slice(core * QS, (core + 1) * QS)
        ks = slice(core * D, (core + 1) * D)
        wqT = chunk_pfirst(np.ascontiguousarray(wq[qs].T)).astype(bfl)
        wkvT = chunk_pfirst(
            np.ascontiguousarray(np.concatenate([wk[ks].T, wv[ks].T], axis=1))
        ).astype(bfl)
        wcT = chunk_pfirst(np.ascontiguousarray(wc[:, qs].T)).astype(bfl)
        per_core.append(
            dict(
                xT=xT_h,
                wqT=wqT,
                wkvT=wkvT,
                wcT=wcT,
                cs=cs_h,
                rotT=rot_h,
                idn=idn_h,
                maskpat=mp_h,
            )
        )
    return per_core


def _run(inputs, trace=False):
    import sys

    if "/opt/trn_rl_repo" not in sys.path:
        sys.path.insert(0, "/opt/trn_rl_repo")
    from concourse.bass_utils import run_bass_kernel_spmd

    x = np.asarray(inputs["x"], np.float32)
    wq = np.asarray(inputs["wq"], np.float32)
    wk = np.asarray(inputs["wk"], np.float32)
    wv = np.asarray(inputs["wv"], np.float32)
    wc = np.asarray(inputs["wc"], np.float32)

    if "nc" not in _CACHE:
        _CACHE["nc"] = _build_program()
    nc = _CACHE["nc"]

    in_maps = _host_inputs(x, wq, wk, wv, wc)
    br = run_bass_kernel_spmd(nc, in_maps, list(range(NCORES)), trace=trace)

    out = np.zeros((BT, C), np.float32)
    for r in br.results:
        out += np.asarray(r["partial"], dtype=np.float32)
    return out.reshape(B, T, C), br


def kernel(**inputs):
    out, _ = _run(inputs, trace=False)
    return out
